# revision 53
# baseline (speedup 1.0000x reference)
"""Trainium2 Bass kernel for nn_CorrTrajBlock (sparse_attention).

Data-parallel over batch B=8 across 8 NeuronCores; one sample per core.

v2: f32r matmuls (1 cyc/row at free>=256), transposed bf16 traj gathers
(no PE transposes for traj/points), no x reload in the main loop (zT
lhsT = SBUF xc bitcast), +x fused into the PSUM->SBUF copy, per-frame
pipelining of affinity/topk/gather/fuse under the x HBM load, and a
software-pipelined (skew=2) zT/softmax/transpose/prop main loop.

Per-core pipeline (C=512, T=8, H=W=28, HW=784, S=T*HW=6272, R=64, K=4,
Cq=128, P=T*R=512):
  1. template_p = w_reduce_eff @ x[:, 0]           (f32r matmul, 64x784)
     spt_inds   = argmax over HW                   (DVE max/max_index)
  2. template_resample = gather cols of x frame 0  (dma_gather f32)
  3. per pi (2 frames): affinity = tres^T @ x_t (f32r), top4 (DVE),
     stage idxs, bf16 transposed traj gathers, fuse matmuls, points adds
  4. coords matmuls + fuse-max + bias2; conv_t + bias3 + relu -> tc
  5. zT[s,p] = x^T @ points (f32r); softmax over p; PE-transpose (bf16)
  6. prop = tcT^T @ proj (bf16); out = prop + x (DVE/GpSimd add) -> DRAM
"""
import sys

sys.path.insert(0, "/opt/trn_rl_repo")

import numpy as np
import concourse.bass as bass
import concourse.mybir as mybir
import concourse.tile as tile
from concourse import bacc
from concourse.bass_utils import run_bass_kernel_spmd

F32 = mybir.dt.float32
F32R = mybir.dt.float32r
BF16 = mybir.dt.bfloat16
I16 = mybir.dt.int16
I32 = mybir.dt.int32
U32 = mybir.dt.uint32
AF = mybir.ActivationFunctionType
ALU = mybir.AluOpType
AX = mybir.AxisListType

B, C, T, H, W = 8, 512, 8, 28, 28
HW = H * W            # 784
S = T * HW            # 6272
R = 64
K = 4
Cq = 128
P = T * R             # 512
CC = C // 128         # 4
NST = S // 128        # 49 s-tiles

_CACHED = {}


def build_nc():
    nc = bacc.Bacc("TRN2", debug=False)

    X_CS = nc.dram_tensor("x_cs", [C, S], F32, kind="ExternalInput").ap()
    X_SB = nc.dram_tensor("x_sc_bf", [S, C], BF16, kind="ExternalInput").ap()
    IOTA7 = nc.dram_tensor("iota7", [128, 7], F32, kind="ExternalInput").ap()
    ONES1 = nc.dram_tensor("ones1", [1, 128], F32, kind="ExternalInput").ap()
    WRT = nc.dram_tensor("wrT", [C, R], F32, kind="ExternalInput").ap()
    WPT = nc.dram_tensor("wpT", [C, Cq], BF16, kind="ExternalInput").ap()
    WPC = nc.dram_tensor("wpc", [2, Cq], BF16, kind="ExternalInput").ap()
    WTT = nc.dram_tensor("wtT", [3, Cq, C], BF16, kind="ExternalInput").ap()
    B2 = nc.dram_tensor("b2", [Cq, 1], F32, kind="ExternalInput").ap()
    B3 = nc.dram_tensor("b3", [CC, 128], F32, kind="ExternalInput").ap()
    IDF = nc.dram_tensor("identf", [128, 128], F32, kind="ExternalInput").ap()
    IDB = nc.dram_tensor("identbf", [128, 128], BF16, kind="ExternalInput").ap()
    OUT = nc.dram_tensor("out", [C, S], F32, kind="ExternalOutput").ap()

    # DRAM scratch for traj-gather index staging (partition rearrangement)
    GREP = nc.dram_tensor("grep_scr", [2048], I16, kind="ExternalOutput").ap()
    CROW = nc.dram_tensor("crow_scr", [2048], BF16, kind="ExternalOutput").ap()
    CCOL = nc.dram_tensor("ccol_scr", [2048], BF16, kind="ExternalOutput").ap()

    Xr = X_CS.rearrange("(cc p) s -> p cc s", p=128)
    OUTr = OUT.rearrange("(cc p) s -> p cc s", p=128)

    with tile.TileContext(nc) as tc:
        import contextlib
        ctx = contextlib.ExitStack()
        pers = ctx.enter_context(tc.tile_pool(name="pers", bufs=1))
        sb = ctx.enter_context(tc.tile_pool(name="sb", bufs=2))
        sb3 = ctx.enter_context(tc.tile_pool(name="sb3", bufs=3))
        sb4 = ctx.enter_context(tc.tile_pool(name="sb4", bufs=4))
        gkp = ctx.enter_context(tc.tile_pool(name="gkp", bufs=8))
        ps = ctx.enter_context(tc.tile_pool(name="ps", bufs=4, space="PSUM"))
        pstp = ctx.enter_context(tc.tile_pool(name="pstp", bufs=4, space="PSUM"))

        # ---- persistent loads ----
        # xc is F32R-labeled so it can feed f32r matmuls directly; the +x
        # add reads it back as plain f32 via bitcast (same bits).
        Xrr = Xr.bitcast(F32R)
        xcr = pers.tile([128, CC, S], F32R, tag="xc")
        nc.sync.dma_start(out=xcr[:, :, 0:HW], in_=Xrr[:, :, 0:HW])  # frame 0
        xc = xcr.bitcast(F32)

        wrT_r = pers.tile([128, CC, R], F32R, tag="wrT")
        nc.sync.dma_start(out=wrT_r,
                          in_=WRT.bitcast(F32R).rearrange("(cc p) r -> p cc r",
                                                          p=128))
        iota7_t = pers.tile([128, 7], F32, tag="iota7")
        nc.sync.dma_start(out=iota7_t, in_=IOTA7)
        ones1_t = pers.tile([1, 128], F32R, tag="ones1")
        nc.sync.dma_start(out=ones1_t, in_=ONES1.bitcast(F32R))
        wpT_t = pers.tile([128, CC, Cq], BF16, tag="wpT")
        nc.sync.dma_start(out=wpT_t, in_=WPT.rearrange("(cc p) q -> p cc q", p=128))
        wpc_t = pers.tile([2, Cq], BF16, tag="wpc")
        nc.sync.dma_start(out=wpc_t, in_=WPC)
        wtT_t = pers.tile([128, 3, C], BF16, tag="wtT")
        nc.sync.dma_start(out=wtT_t, in_=WTT.rearrange("d p c -> p d c"))
        b2_t = pers.tile([128, 1], F32, tag="b2")
        nc.sync.dma_start(out=b2_t, in_=B2)
        b3_t = pers.tile([128, CC], F32, tag="b3")
        nc.sync.dma_start(out=b3_t, in_=B3.rearrange("cc p -> p cc"))
        idf_t = pers.tile([128, 128], F32, tag="idf")
        nc.sync.dma_start(out=idf_t, in_=IDF)
        idb_t = pers.tile([128, 128], BF16, tag="idb")
        nc.sync.dma_start(out=idb_t, in_=IDB)

        # rest of x: per-t-block DMAs so affinity can stream per t; the
        # last two frames are chunked per cc so the final queue drain is
        # granular (staging round trips interleave sooner)
        for tb in range(1, 6):
            nc.sync.dma_start(out=xcr[:, :, tb * HW:(tb + 1) * HW],
                              in_=Xrr[:, :, tb * HW:(tb + 1) * HW])
        for tb in range(6, T):
            for cc in range(CC):
                nc.sync.dma_start(
                    out=xcr[:, cc:cc + 1, tb * HW:(tb + 1) * HW],
                    in_=Xrr[:, cc:cc + 1, tb * HW:(tb + 1) * HW])

        # ---- phase 1: template (frame 0 only) ----
        tpl_sb = pers.tile([64, HW], F32, tag="tpl")
        for h in range(2):
            tp_ps = ps.tile([64, 392], F32, tag="acc")
            for cc in range(CC):
                nc.tensor.matmul(tp_ps, lhsT=wrT_r[:, cc, :],
                                 rhs=xcr[:, cc, h * 392:(h + 1) * 392],
                                 start=(cc == 0), stop=(cc == CC - 1))
            nc.scalar.activation(tpl_sb[:, h * 392:(h + 1) * 392], tp_ps, AF.Copy)
        tmx = pers.tile([64, 8], F32, tag="tmx")
        tmi = pers.tile([64, 8], U32, tag="tmi")
        nc.vector.max(out=tmx, in_=tpl_sb)
        nc.vector.max_index(out=tmi, in_max=tmx, in_values=tpl_sb)
        # frame 0 of x in (s, c) layout via PE transposes of xc (no extra
        # HBM load); hb=6 reads into frame 1 (s 768..895) but the one-hot
        # zeroes those columns
        xsc0 = pers.tile([128, 7, C], F32R, tag="xsc0")
        for hb in range(7):
            for cc in range(CC):
                tp0 = pstp.tile([128, 128], F32, tag="tp")
                nc.tensor.transpose(
                    tp0, xc[:, cc, 128 * hb:128 * (hb + 1)], idf_t)
                nc.scalar.activation(xsc0[:, hb, cc * 128:(cc + 1) * 128],
                                     tp0, AF.Copy)
        # on-chip template resample: tresT = onehot(spt)^T @ x_sc0, with no
        # DMA round trip (spt broadcast via PE, one-hot via iota compare,
        # resample via 7 f32r matmuls over s-blocks)
        sptf = pers.tile([64, 1], F32, tag="sptf")
        nc.vector.tensor_copy(sptf, tmi[:, 0:1])
        spt_tp = pstp.tile([1, 64], F32, tag="tp")
        nc.tensor.transpose(spt_tp, sptf, idf_t[0:64, 0:64])
        spt_row = pers.tile([1, 64], F32R, tag="sptrow")
        nc.scalar.activation(spt_row, spt_tp, AF.Copy)
        bc_ps = pstp.tile([128, 64], F32, tag="tp")
        nc.tensor.matmul(bc_ps, lhsT=ones1_t, rhs=spt_row)
        spt_bc = pers.tile([128, 64], F32, tag="sptbc")
        nc.scalar.activation(spt_bc, bc_ps, AF.Copy)
        oh7 = pers.tile([128, 7, 64], F32R, tag="oh7")
        for hb in range(7):
            nc.vector.tensor_scalar(oh7[:, hb, :], spt_bc,
                                    iota7_t[:, hb:hb + 1], None, op0=ALU.is_equal)
        trT_ps = ps.tile([64, C], F32, tag="acc")
        for hb in range(7):
            nc.tensor.matmul(trT_ps, lhsT=oh7[:, hb, :], rhs=xsc0[:, hb, :],
                             start=(hb == 0), stop=(hb == 6))
        tresT = pers.tile([64, C], F32, tag="tresT")
        nc.scalar.activation(tresT, trT_ps, AF.Copy)
        # tres2[:, tt, cc, 64*tt : 64*tt+64] holds the template block, rest
        # zero: two frames accumulate into one [128, 392] PSUM without
        # tile_position (invalid for f32r matmuls).
        tres2 = pers.tile([128, 2, CC, 128], F32R, tag="tres2")
        for cc in range(CC):
            tp = pstp.tile([128, 128], F32, tag="tp")
            nc.tensor.transpose(tp[:, 0:64],
                                tresT[:, cc * 128:(cc + 1) * 128],
                                idf_t[0:64, 0:64])
            nc.scalar.activation(tres2[:, 0, cc, 0:64], tp[:, 0:64], AF.Copy)
            nc.scalar.activation(tres2[:, 0, cc, 64:128], tp[:, 0:64], AF.Copy,
                                 scale=0.0)
            nc.scalar.activation(tres2[:, 1, cc, 0:64], tp[:, 0:64], AF.Copy,
                                 scale=0.0)
            nc.scalar.activation(tres2[:, 1, cc, 64:128], tp[:, 0:64], AF.Copy)

        # ---- phases 2+3 pipelined per pi (2 frames each) ----
        gstage = pers.tile([128, 16], I16, tag="gstage")
        fstage = pers.tile([128, 16], F32, tag="fstage")
        gs_v = gstage.rearrange("p (k pi) -> p pi k", pi=4)
        fs_v = fstage.rearrange("p (k pi) -> p pi k", pi=4)
        rowst = pers.tile([128, 16], BF16, tag="rowst")
        colst = pers.tile([128, 16], BF16, tag="colst")
        row_v = rowst.rearrange("p (k pi) -> p pi k", pi=4)
        col_v = colst.rearrange("p (k pi) -> p pi k", pi=4)
        # staging layouts: flat j = (k*4+pi)*128 + p, matching the SBUF
        # staging tiles' (k pi) column order so writes balance trivially
        CROWv = CROW.rearrange("(q p) -> p q", p=128)
        CCOLv = CCOL.rearrange("(q p) -> p q", p=128)
        CROWb = CROW.rearrange("(k pi p) -> pi k p", pi=4, p=128)
        CCOLb = CCOL.rearrange("(k pi p) -> pi k p", pi=4, p=128)
        GIDXv = GREP.rearrange("(q p) -> p q", p=128)
        GIDXw = GREP.rearrange("(km c) -> c km", c=16, km=128)

        pts_r = pers.tile([128, CC, P], F32R, tag="pts")
        fm_f32 = pers.tile([128, P], F32, tag="fmf")
        gk_all = [[None] * K for _ in range(4)]

        def emit_affinity_topk_stage(pi):
            aff_sb = sb.tile([128, HW], F32, tag="aff")
            for h in range(2):
                a_ps = ps.tile([128, 392], F32, tag="acc")
                for tt in range(2):
                    t = 2 * pi + tt
                    for cc in range(CC):
                        nc.tensor.matmul(
                            a_ps,
                            lhsT=tres2[:, tt, cc, :],
                            rhs=xcr[:, cc, t * HW + h * 392: t * HW + (h + 1) * 392],
                            start=(tt == 0 and cc == 0),
                            stop=(tt == 1 and cc == CC - 1))
                nc.scalar.activation(aff_sb[:, h * 392:(h + 1) * 392], a_ps, AF.Copy)
            amx = sb.tile([128, 8], F32, tag="amx")
            ami = sb.tile([128, 8], U32, tag="ami")
            nc.vector.max(out=amx, in_=aff_sb)
            nc.vector.max_index(out=ami, in_max=amx, in_values=aff_sb)
            for tt in range(2):
                t = 2 * pi + tt
                rows = slice(64 * tt, 64 * (tt + 1))
                nc.vector.tensor_scalar(gs_v[rows, pi, :],
                                        ami[rows, 0:K], float(t * HW), None,
                                        op0=ALU.add)
            nc.vector.tensor_copy(fs_v[:, pi, :], ami[:, 0:K])
            # coords: row=(i//28)/28, col=i/28-(i//28); robust floor
            vq = sb.tile([128, 4], F32, tag="vq")
            nc.vector.tensor_scalar(vq, fs_v[:, pi, :], 1.0 / 28.0, None,
                                    op0=ALU.mult)
            qi = sb.tile([128, 4], I32, tag="qi")
            nc.vector.tensor_copy(qi, vq)
            qf = sb.tile([128, 4], F32, tag="qf")
            nc.vector.tensor_copy(qf, qi)
            cgt = sb.tile([128, 4], F32, tag="cgt")
            nc.vector.tensor_tensor(out=cgt, in0=qf, in1=vq, op=ALU.is_gt)
            nc.vector.tensor_tensor(out=qf, in0=qf, in1=cgt, op=ALU.subtract)
            nc.vector.tensor_scalar(row_v[:, pi, :], qf, 1.0 / 28.0, None,
                                    op0=ALU.mult)
            nc.vector.scalar_tensor_tensor(col_v[:, pi, :], in0=fs_v[:, pi, :],
                                           scalar=1.0 / 28.0, in1=qf,
                                           op0=ALU.mult, op1=ALU.subtract)

        def emit_fuse_pts(pi):
            gks = gk_all[pi]
            # per-pi coords slices (depend on this pi's CROW/CCOL writes)
            cpi = sb.tile([2, K, 128], BF16, tag="cpi")
            nc.scalar.dma_start(out=cpi[0:1, :, :], in_=CROWb[pi])
            nc.scalar.dma_start(out=cpi[1:2, :, :], in_=CCOLb[pi])
            fps_k = []
            for k in range(K):
                f_ps = pstp.tile([128, 128], F32, tag="tp")
                for cc in range(CC):
                    nc.tensor.matmul(f_ps, lhsT=wpT_t[:, cc, :],
                                     rhs=gks[k][:, cc, :],
                                     start=(cc == 0), stop=False)
                nc.tensor.matmul(f_ps, lhsT=wpc_t, rhs=cpi[:, k, :],
                                 start=False, stop=True)
                fps_k.append(f_ps)
            # fm slice = max over k (one PSUM input per op)
            fsl = fm_f32[:, pi * 128:(pi + 1) * 128]
            nc.scalar.activation(fsl, fps_k[0], AF.Copy)
            for kk in range(1, 4):
                nc.vector.tensor_tensor(out=fsl, in0=fsl, in1=fps_k[kk],
                                        op=ALU.max)
            # points slice: sum of 4 bf16 gathers in f32
            a01 = sb.tile([128, CC, 128], F32, tag="a01")
            nc.vector.tensor_tensor(out=a01, in0=gks[0], in1=gks[1], op=ALU.add)
            a23 = sb.tile([128, CC, 128], F32, tag="a23")
            nc.gpsimd.tensor_tensor(out=a23, in0=gks[2], in1=gks[3], op=ALU.add)
            nc.vector.tensor_tensor(out=pts_r[:, :, pi * 128:(pi + 1) * 128],
                                    in0=a01, in1=a23, op=ALU.add)

        # affinity/topk per pi; staging + gathers in two pi-pair batches so
        # gather descgen/data hide under the x-load tail; fuse last
        g2all = pers.tile([128, 128], I16, tag="g2all")

        gs_kv = gstage.rearrange("p (k pi) -> p k pi", pi=4)
        row_kv = rowst.rearrange("p (k pi) -> p k pi", pi=4)
        col_kv = colst.rearrange("p (k pi) -> p k pi", pi=4)
        GIkv = GREP.rearrange("(k pi p) -> p k pi", pi=4, p=128)
        CRkv = CROW.rearrange("(k pi p) -> p k pi", pi=4, p=128)
        CCkv = CCOL.rearrange("(k pi p) -> p k pi", pi=4, p=128)

        def emit_stage_gather(plo):
            pis = [plo, plo + 1]
            for pi in pis:
                nc.gpsimd.dma_start(out=GIkv[:, :, pi], in_=gs_kv[:, :, pi])
                nc.scalar.dma_start(out=CRkv[:, :, pi], in_=row_kv[:, :, pi])
                nc.scalar.dma_start(out=CCkv[:, :, pi], in_=col_kv[:, :, pi])
            # full-width replica reads; the other pair's columns are stale
            # here but unused until batch 2 rewrites and rereads them
            for g in range(8):
                nc.gpsimd.dma_start(out=g2all[16 * g:16 * (g + 1), :],
                                    in_=GIDXw)
            for pi in pis:
                for k in range(K):
                    gkT = gkp.tile([128, CC, 128], BF16, tag="gk")
                    blk = (4 * k + pi) * 8
                    nc.gpsimd.dma_gather(out_ap=gkT, in_ap=X_SB,
                                         idxs_ap=g2all[:, blk:blk + 8],
                                         num_idxs=128, num_idxs_reg=128,
                                         elem_size=C, transpose=True)
                    gk_all[pi][k] = gkT

        emit_affinity_topk_stage(0)
        emit_affinity_topk_stage(1)
        emit_stage_gather(0)
        emit_affinity_topk_stage(2)
        emit_affinity_topk_stage(3)
        emit_stage_gather(2)
        for pi in range(4):
            emit_fuse_pts(pi)

        fm = pers.tile([128, P], BF16, tag="fm")
        nc.vector.tensor_scalar(fm, fm_f32, b2_t, None, op0=ALU.add)

        # conv over t (3 taps) + bias3 + relu -> tc bf16
        tc_bf = pers.tile([128, CC, P], BF16, tag="tcbf")
        for ct in range(CC):
            c_ps = ps.tile([128, P], F32, tag="acc")
            cs = slice(ct * 128, (ct + 1) * 128)
            nc.tensor.matmul(c_ps, lhsT=wtT_t[:, 1, cs], rhs=fm,
                             start=True, stop=False)
            nc.tensor.matmul(c_ps[:, R:P], lhsT=wtT_t[:, 0, cs], rhs=fm[:, 0:P - R],
                             start=False, stop=False)
            nc.tensor.matmul(c_ps[:, 0:P - R], lhsT=wtT_t[:, 2, cs], rhs=fm[:, R:P],
                             start=False, stop=True)
            nc.scalar.activation(tc_bf[:, ct, :], c_ps, AF.Relu,
                                 bias=b3_t[:, ct:ct + 1])
        tcT = pers.tile([128, CC, C], BF16, tag="tcT")
        for pb in range(4):
            tp2 = pstp.tile([128, C], BF16, tag="tp")
            for cc in range(CC):
                nc.tensor.transpose(tp2[:, cc * 128:(cc + 1) * 128],
                                    tc_bf[:, cc, pb * 128:(pb + 1) * 128], idb_t)
            nc.vector.tensor_copy(tcT[:, pb, :], tp2)

        # ---- phases 4+5: software-pipelined (skew=2) main loop ----
        SKEW = 3
        pj_tiles = [None] * NST
        projTP = {}

        def emit_zT_softmax(st):
            z_ps = ps.tile([128, P], F32, tag="acc")
            for cc in range(CC):
                nc.tensor.matmul(z_ps,
                                 lhsT=xcr[:, cc, st * 128:(st + 1) * 128],
                                 rhs=pts_r[:, cc, :],
                                 start=(cc == 0), stop=(cc == CC - 1))
            nm = sb3.tile([128, 1], F32, tag="nm")
            nc.vector.tensor_reduce(nm, z_ps, axis=AX.X, op=ALU.max, negate=True)
            nm4 = sb3.tile([128, 1], F32, tag="nm4")
            nc.vector.tensor_scalar(nm4, nm, 0.25, None, op0=ALU.mult)
            e_sb = sb3.tile([128, P], F32, tag="esb")
            dsum = sb3.tile([128, 1], F32, tag="dsum")
            nc.scalar.activation(e_sb, z_ps, AF.Exp, bias=nm4, scale=0.25,
                                 accum_out=dsum)
            rd = sb3.tile([128, 1], F32, tag="rd")
            nc.vector.reciprocal(rd, dsum)
            pjT = sb4.tile([128, P], BF16, tag="pjT")
            nc.vector.tensor_scalar(pjT, e_sb, rd, None, op0=ALU.mult)
            pj_tiles[st] = pjT

        def emit_transposes(st):
            chunk, slot = st // 4, st % 4
            if slot == 0:
                tiles = []
                for _pb in range(4):
                    pjt_tile = pstp.tile([128, P], BF16, tag="tp")
                    tiles.append(pjt_tile)
                projTP[chunk] = tiles
            tgt = projTP[chunk]
            pjT = pj_tiles[st]
            for pb in range(4):
                nc.tensor.transpose(tgt[pb][:, slot * 128:(slot + 1) * 128],
                                    pjT[:, pb * 128:(pb + 1) * 128], idb_t)

        def emit_prop(chunk):
            nslots = min(4, NST - chunk * 4)
            cw = nslots * 128
            tgt = projTP[chunk]
            proj_ch = sb.tile([128, 4, P], BF16, tag="projch")
            for pb in range(4):
                if pb < 2:
                    nc.scalar.activation(proj_ch[:, pb, 0:cw],
                                         tgt[pb][:, 0:cw], AF.Copy)
                else:
                    nc.vector.tensor_copy(proj_ch[:, pb, 0:cw],
                                          tgt[pb][:, 0:cw])
            for ct in range(CC):
                p_ps = pstp.tile([128, cw], F32, tag="tp")
                for pb in range(4):
                    nc.tensor.matmul(p_ps, lhsT=tcT[:, pb, ct * 128:(ct + 1) * 128],
                                     rhs=proj_ch[:, pb, 0:cw],
                                     start=(pb == 0), stop=(pb == 3))
                osb = sb4.tile([128, cw], F32, tag="osb")
                xsl = xc[:, ct, chunk * P:chunk * P + cw]
                if ct % 2 == 0:
                    nc.vector.tensor_tensor(out=osb, in0=p_ps, in1=xsl,
                                            op=ALU.add)
                else:
                    # GpSimd can't read PSUM: ACT copies, GpSimd adds in SBUF
                    nc.scalar.activation(osb, p_ps, AF.Copy)
                    nc.gpsimd.tensor_tensor(out=osb, in0=osb, in1=xsl,
                                            op=ALU.add)
                nc.sync.dma_start(out=OUTr[:, ct, chunk * P:chunk * P + cw],
                                  in_=osb)

        for st in range(NST):
            emit_zT_softmax(st)
            if st >= SKEW:
                emit_transposes(st - SKEW)
                if (st - SKEW) % 4 == 3:
                    emit_prop((st - SKEW) // 4)
        for st in range(NST - SKEW, NST):
            emit_transposes(st)
            if st % 4 == 3 or st == NST - 1:
                emit_prop(st // 4)
        ctx.close()
    nc.compile()
    return nc


def _host_prep(inputs):
    eps = 1e-5
    f32 = np.float32
    x = np.asarray(inputs["input"], f32)                       # (B,C,T,H,W)
    s1 = np.asarray(inputs["bn1_gamma"]) / np.sqrt(np.asarray(inputs["bn1_var"]) + eps)
    wrT = (np.asarray(inputs["w_reduce"], f32) * s1[:, None]).T.astype(f32)
    s2 = np.asarray(inputs["bn2_gamma"]) / np.sqrt(np.asarray(inputs["bn2_var"]) + eps)
    wp = np.asarray(inputs["w_proj"], f32) * s2[:, None]       # (Cq, C+2)
    b2 = (np.asarray(inputs["bn2_beta"])
          - np.asarray(inputs["bn2_mean"]) * s2).astype(f32)
    s3 = np.asarray(inputs["bn3_gamma"]) / np.sqrt(np.asarray(inputs["bn3_var"]) + eps)
    wt = np.asarray(inputs["w_t"], f32)[:, :, :, 0] * s3[:, None, None]  # (C,Cq,3)
    b3 = (np.asarray(inputs["bn3_beta"])
          - np.asarray(inputs["bn3_mean"]) * s3).astype(f32)
    import ml_dtypes
    bf16 = ml_dtypes.bfloat16
    common = {
        "wrT": np.ascontiguousarray(wrT),
        "wpT": np.ascontiguousarray(wp[:, :C].T.astype(bf16)),
        "wpc": np.ascontiguousarray(wp[:, C:].T.astype(bf16)),
        "wtT": np.ascontiguousarray(np.transpose(wt, (2, 1, 0)).astype(bf16)),
        "b2": b2.reshape(Cq, 1),
        "b3": b3.reshape(CC, 128),
        "identf": np.eye(128, dtype=f32),
        "identbf": np.eye(128, dtype=bf16),
    }
    iota = np.arange(128, dtype=f32)
    common["iota7"] = np.ascontiguousarray(
        np.stack([iota + 128 * hb for hb in range(7)], axis=1))
    common["ones1"] = np.ones((1, 128), f32)
    x_cs = x.reshape(B, C, S)
    x_sc = np.ascontiguousarray(np.transpose(x_cs, (0, 2, 1)))
    x_sc_bf = x_sc.astype(bf16)
    x_sc0 = np.zeros((B, 896, C), f32)
    x_sc0[:, 0:HW] = x_sc[:, 0:HW]
    in_maps = []
    for b in range(B):
        m = dict(common)
        m["x_cs"] = np.ascontiguousarray(x_cs[b])
        m["x_sc0"] = x_sc0[b]
        m["x_sc_bf"] = x_sc_bf[b]
        in_maps.append(m)
    return in_maps


def kernel(**inputs) -> np.ndarray:
    if "nc" not in _CACHED:
        _CACHED["nc"] = build_nc()
    nc = _CACHED["nc"]
    in_maps = _host_prep(inputs)
    res = run_bass_kernel_spmd(nc, in_maps, list(range(B)))
    out = np.stack([res.results[b]["out"] for b in range(B)], axis=0)
    return out.reshape(B, C, T, H, W).astype(np.float32)


if __name__ == "__main__":
    pass


# revision 54
# speedup vs baseline: 1.5247x; 1.5247x over previous
"""Trainium2 Bass kernel for nn_CorrTrajBlock (sparse_attention).

Data-parallel over batch B=8 across 8 NeuronCores; one sample per core.

v2: f32r matmuls (1 cyc/row at free>=256), transposed bf16 traj gathers
(no PE transposes for traj/points), no x reload in the main loop (zT
lhsT = SBUF xc bitcast), +x fused into the PSUM->SBUF copy, per-frame
pipelining of affinity/topk/gather/fuse under the x HBM load, and a
software-pipelined (skew=2) zT/softmax/transpose/prop main loop.

Per-core pipeline (C=512, T=8, H=W=28, HW=784, S=T*HW=6272, R=64, K=4,
Cq=128, P=T*R=512):
  1. template_p = w_reduce_eff @ x[:, 0]           (f32r matmul, 64x784)
     spt_inds   = argmax over HW                   (DVE max/max_index)
  2. template_resample = gather cols of x frame 0  (dma_gather f32)
  3. per pi (2 frames): affinity = tres^T @ x_t (f32r), top4 (DVE),
     stage idxs, bf16 transposed traj gathers, fuse matmuls, points adds
  4. coords matmuls + fuse-max + bias2; conv_t + bias3 + relu -> tc
  5. zT[s,p] = x^T @ points (f32r); softmax over p; PE-transpose (bf16)
  6. prop = tcT^T @ proj (bf16); out = prop + x (DVE/GpSimd add) -> DRAM
"""
import sys

sys.path.insert(0, "/opt/trn_rl_repo")

import numpy as np
import concourse.bass as bass
import concourse.mybir as mybir
import concourse.tile as tile
from concourse import bacc
from concourse.bass_utils import run_bass_kernel_spmd

F32 = mybir.dt.float32
F32R = mybir.dt.float32r
BF16 = mybir.dt.bfloat16
I16 = mybir.dt.int16
I32 = mybir.dt.int32
U32 = mybir.dt.uint32
AF = mybir.ActivationFunctionType
ALU = mybir.AluOpType
AX = mybir.AxisListType

B, C, T, H, W = 8, 512, 8, 28, 28
HW = H * W            # 784
S = T * HW            # 6272
R = 64
K = 4
Cq = 128
P = T * R             # 512
CC = C // 128         # 4
NST = S // 128        # 49 s-tiles

_CACHED = {}


def build_nc():
    nc = bacc.Bacc("TRN2", debug=False)

    X_CS = nc.dram_tensor("x_cs", [C, S], F32, kind="ExternalInput").ap()
    X_SB = nc.dram_tensor("x_sc_bf", [S, C], BF16, kind="ExternalInput").ap()
    IOTA7 = nc.dram_tensor("iota7", [128, 7], F32, kind="ExternalInput").ap()
    ONES1 = nc.dram_tensor("ones1", [1, 128], F32, kind="ExternalInput").ap()
    WRT = nc.dram_tensor("wrT", [C, R], F32, kind="ExternalInput").ap()
    WPT = nc.dram_tensor("wpT", [C, Cq], BF16, kind="ExternalInput").ap()
    WPC = nc.dram_tensor("wpc", [2, Cq], BF16, kind="ExternalInput").ap()
    WTT = nc.dram_tensor("wtT", [3, Cq, C], BF16, kind="ExternalInput").ap()
    B2 = nc.dram_tensor("b2", [Cq, 1], F32, kind="ExternalInput").ap()
    B3 = nc.dram_tensor("b3", [CC, 128], F32, kind="ExternalInput").ap()
    IDF = nc.dram_tensor("identf", [128, 128], F32, kind="ExternalInput").ap()
    IDB = nc.dram_tensor("identbf", [128, 128], BF16, kind="ExternalInput").ap()
    OUT = nc.dram_tensor("out", [C, S], F32, kind="ExternalOutput").ap()

    # DRAM scratch for traj-gather index staging (partition rearrangement)
    GREP = nc.dram_tensor("grep_scr", [2048], I16, kind="ExternalOutput").ap()
    CROW = nc.dram_tensor("crow_scr", [2048], BF16, kind="ExternalOutput").ap()
    CCOL = nc.dram_tensor("ccol_scr", [2048], BF16, kind="ExternalOutput").ap()

    Xr = X_CS.rearrange("(cc p) s -> p cc s", p=128)
    OUTr = OUT.rearrange("(cc p) s -> p cc s", p=128)

    with tile.TileContext(nc) as tc:
        import contextlib
        ctx = contextlib.ExitStack()
        pers = ctx.enter_context(tc.tile_pool(name="pers", bufs=1))
        sb = ctx.enter_context(tc.tile_pool(name="sb", bufs=2))
        sb3 = ctx.enter_context(tc.tile_pool(name="sb3", bufs=3))
        sb4 = ctx.enter_context(tc.tile_pool(name="sb4", bufs=4))
        gkp = ctx.enter_context(tc.tile_pool(name="gkp", bufs=8))
        ps = ctx.enter_context(tc.tile_pool(name="ps", bufs=4, space="PSUM"))
        pstp = ctx.enter_context(tc.tile_pool(name="pstp", bufs=4, space="PSUM"))

        # ---- persistent loads ----
        # xc is F32R-labeled so it can feed f32r matmuls directly; the +x
        # add reads it back as plain f32 via bitcast (same bits).
        Xrr = Xr.bitcast(F32R)
        xcr = pers.tile([128, CC, S], F32R, tag="xc")
        nc.sync.dma_start(out=xcr[:, :, 0:HW], in_=Xrr[:, :, 0:HW])  # frame 0
        xc = xcr.bitcast(F32)

        wrT_r = pers.tile([128, CC, R], F32R, tag="wrT")
        nc.sync.dma_start(out=wrT_r,
                          in_=WRT.bitcast(F32R).rearrange("(cc p) r -> p cc r",
                                                          p=128))
        iota7_t = pers.tile([128, 7], F32, tag="iota7")
        nc.sync.dma_start(out=iota7_t, in_=IOTA7)
        ones1_t = pers.tile([1, 128], F32R, tag="ones1")
        nc.sync.dma_start(out=ones1_t, in_=ONES1.bitcast(F32R))
        wpT_t = pers.tile([128, CC, Cq], BF16, tag="wpT")
        nc.sync.dma_start(out=wpT_t, in_=WPT.rearrange("(cc p) q -> p cc q", p=128))
        wpc_t = pers.tile([2, Cq], BF16, tag="wpc")
        nc.sync.dma_start(out=wpc_t, in_=WPC)
        wtT_t = pers.tile([128, 3, C], BF16, tag="wtT")
        nc.sync.dma_start(out=wtT_t, in_=WTT.rearrange("d p c -> p d c"))
        b2_t = pers.tile([128, 1], F32, tag="b2")
        nc.sync.dma_start(out=b2_t, in_=B2)
        b3_t = pers.tile([128, CC], F32, tag="b3")
        nc.sync.dma_start(out=b3_t, in_=B3.rearrange("cc p -> p cc"))
        idf_t = pers.tile([128, 128], F32, tag="idf")
        nc.sync.dma_start(out=idf_t, in_=IDF)
        idb_t = pers.tile([128, 128], BF16, tag="idb")
        nc.sync.dma_start(out=idb_t, in_=IDB)

        # rest of x: per-t-block DMAs so affinity can stream per t
        for tb in range(1, T):
            nc.sync.dma_start(out=xcr[:, :, tb * HW:(tb + 1) * HW],
                              in_=Xrr[:, :, tb * HW:(tb + 1) * HW])

        # ---- phase 1: template (frame 0 only) ----
        tpl_sb = pers.tile([64, HW], F32, tag="tpl")
        for h in range(2):
            tp_ps = ps.tile([64, 392], F32, tag="acc")
            for cc in range(CC):
                nc.tensor.matmul(tp_ps, lhsT=wrT_r[:, cc, :],
                                 rhs=xcr[:, cc, h * 392:(h + 1) * 392],
                                 start=(cc == 0), stop=(cc == CC - 1))
            nc.scalar.activation(tpl_sb[:, h * 392:(h + 1) * 392], tp_ps, AF.Copy)
        tmx = pers.tile([64, 8], F32, tag="tmx")
        tmi = pers.tile([64, 8], U32, tag="tmi")
        nc.vector.max(out=tmx, in_=tpl_sb)
        nc.vector.max_index(out=tmi, in_max=tmx, in_values=tpl_sb)
        # frame 0 of x in (s, c) layout via PE transposes of xc (no extra
        # HBM load); hb=6 reads into frame 1 (s 768..895) but the one-hot
        # zeroes those columns
        xsc0 = pers.tile([128, 7, C], F32R, tag="xsc0")
        for hb in range(7):
            for cc in range(CC):
                tp0 = pstp.tile([128, 128], F32, tag="tp")
                nc.tensor.transpose(
                    tp0, xc[:, cc, 128 * hb:128 * (hb + 1)], idf_t)
                nc.scalar.activation(xsc0[:, hb, cc * 128:(cc + 1) * 128],
                                     tp0, AF.Copy)
        # on-chip template resample: tresT = onehot(spt)^T @ x_sc0, with no
        # DMA round trip (spt broadcast via PE, one-hot via iota compare,
        # resample via 7 f32r matmuls over s-blocks)
        sptf = pers.tile([64, 1], F32, tag="sptf")
        nc.vector.tensor_copy(sptf, tmi[:, 0:1])
        spt_tp = pstp.tile([1, 64], F32, tag="tp")
        nc.tensor.transpose(spt_tp, sptf, idf_t[0:64, 0:64])
        spt_row = pers.tile([1, 64], F32R, tag="sptrow")
        nc.scalar.activation(spt_row, spt_tp, AF.Copy)
        bc_ps = pstp.tile([128, 64], F32, tag="tp")
        nc.tensor.matmul(bc_ps, lhsT=ones1_t, rhs=spt_row)
        spt_bc = pers.tile([128, 64], F32, tag="sptbc")
        nc.scalar.activation(spt_bc, bc_ps, AF.Copy)
        oh7 = pers.tile([128, 7, 64], F32R, tag="oh7")
        for hb in range(7):
            nc.vector.tensor_scalar(oh7[:, hb, :], spt_bc,
                                    iota7_t[:, hb:hb + 1], None, op0=ALU.is_equal)
        trT_ps = ps.tile([64, C], F32, tag="acc")
        for hb in range(7):
            nc.tensor.matmul(trT_ps, lhsT=oh7[:, hb, :], rhs=xsc0[:, hb, :],
                             start=(hb == 0), stop=(hb == 6))
        tresT = pers.tile([64, C], F32, tag="tresT")
        nc.scalar.activation(tresT, trT_ps, AF.Copy)
        # tres2[:, tt, cc, 64*tt : 64*tt+64] holds the template block, rest
        # zero: two frames accumulate into one [128, 392] PSUM without
        # tile_position (invalid for f32r matmuls).
        tres2 = pers.tile([128, 2, CC, 128], F32R, tag="tres2")
        for cc in range(CC):
            tp = pstp.tile([128, 128], F32, tag="tp")
            nc.tensor.transpose(tp[:, 0:64],
                                tresT[:, cc * 128:(cc + 1) * 128],
                                idf_t[0:64, 0:64])
            nc.scalar.activation(tres2[:, 0, cc, 0:64], tp[:, 0:64], AF.Copy)
            nc.scalar.activation(tres2[:, 0, cc, 64:128], tp[:, 0:64], AF.Copy,
                                 scale=0.0)
            nc.scalar.activation(tres2[:, 1, cc, 0:64], tp[:, 0:64], AF.Copy,
                                 scale=0.0)
            nc.scalar.activation(tres2[:, 1, cc, 64:128], tp[:, 0:64], AF.Copy)

        # ---- phases 2+3 pipelined per pi (2 frames each) ----
        gstage = pers.tile([128, 16], I16, tag="gstage")
        fstage = pers.tile([128, 16], F32, tag="fstage")
        gs_v = gstage.rearrange("p (k pi) -> p pi k", pi=4)
        fs_v = fstage.rearrange("p (k pi) -> p pi k", pi=4)
        rowst = pers.tile([128, 16], BF16, tag="rowst")
        colst = pers.tile([128, 16], BF16, tag="colst")
        row_v = rowst.rearrange("p (k pi) -> p pi k", pi=4)
        col_v = colst.rearrange("p (k pi) -> p pi k", pi=4)
        # staging layouts: flat j = (k*4+pi)*128 + p, matching the SBUF
        # staging tiles' (k pi) column order so writes balance trivially
        CROWv = CROW.rearrange("(q p) -> p q", p=128)
        CCOLv = CCOL.rearrange("(q p) -> p q", p=128)
        CROWb = CROW.rearrange("(k pi p) -> pi k p", pi=4, p=128)
        CCOLb = CCOL.rearrange("(k pi p) -> pi k p", pi=4, p=128)
        GIDXv = GREP.rearrange("(q p) -> p q", p=128)
        GIDXw = GREP.rearrange("(km c) -> c km", c=16, km=128)

        pts_r = pers.tile([128, CC, P], F32R, tag="pts")
        fm_f32 = pers.tile([128, P], F32, tag="fmf")
        gk_all = [[None] * K for _ in range(4)]

        def emit_affinity_topk_stage(pi):
            aff_sb = sb.tile([128, HW], F32, tag="aff")
            for h in range(2):
                a_ps = ps.tile([128, 392], F32, tag="acc")
                for tt in range(2):
                    t = 2 * pi + tt
                    for cc in range(CC):
                        nc.tensor.matmul(
                            a_ps,
                            lhsT=tres2[:, tt, cc, :],
                            rhs=xcr[:, cc, t * HW + h * 392: t * HW + (h + 1) * 392],
                            start=(tt == 0 and cc == 0),
                            stop=(tt == 1 and cc == CC - 1))
                nc.scalar.activation(aff_sb[:, h * 392:(h + 1) * 392], a_ps, AF.Copy)
            amx = sb.tile([128, 8], F32, tag="amx")
            ami = sb.tile([128, 8], U32, tag="ami")
            nc.vector.max(out=amx, in_=aff_sb)
            nc.vector.max_index(out=ami, in_max=amx, in_values=aff_sb)
            for tt in range(2):
                t = 2 * pi + tt
                rows = slice(64 * tt, 64 * (tt + 1))
                nc.vector.tensor_scalar(gs_v[rows, pi, :],
                                        ami[rows, 0:K], float(t * HW), None,
                                        op0=ALU.add)
            nc.vector.tensor_copy(fs_v[:, pi, :], ami[:, 0:K])
            # coords: row=(i//28)/28, col=i/28-(i//28); robust floor
            vq = sb.tile([128, 4], F32, tag="vq")
            nc.vector.tensor_scalar(vq, fs_v[:, pi, :], 1.0 / 28.0, None,
                                    op0=ALU.mult)
            qi = sb.tile([128, 4], I32, tag="qi")
            nc.vector.tensor_copy(qi, vq)
            qf = sb.tile([128, 4], F32, tag="qf")
            nc.vector.tensor_copy(qf, qi)
            cgt = sb.tile([128, 4], F32, tag="cgt")
            nc.vector.tensor_tensor(out=cgt, in0=qf, in1=vq, op=ALU.is_gt)
            nc.vector.tensor_tensor(out=qf, in0=qf, in1=cgt, op=ALU.subtract)
            nc.vector.tensor_scalar(row_v[:, pi, :], qf, 1.0 / 28.0, None,
                                    op0=ALU.mult)
            nc.vector.scalar_tensor_tensor(col_v[:, pi, :], in0=fs_v[:, pi, :],
                                           scalar=1.0 / 28.0, in1=qf,
                                           op0=ALU.mult, op1=ALU.subtract)

        def emit_fuse_pts(pi):
            gks = gk_all[pi]
            # per-pi coords slices (depend on this pi's CROW/CCOL writes)
            cpi = sb.tile([2, K, 128], BF16, tag="cpi")
            nc.scalar.dma_start(out=cpi[0:1, :, :], in_=CROWb[pi])
            nc.scalar.dma_start(out=cpi[1:2, :, :], in_=CCOLb[pi])
            fps_k = []
            for k in range(K):
                f_ps = pstp.tile([128, 128], F32, tag="tp")
                for cc in range(CC):
                    nc.tensor.matmul(f_ps, lhsT=wpT_t[:, cc, :],
                                     rhs=gks[k][:, cc, :],
                                     start=(cc == 0), stop=False)
                nc.tensor.matmul(f_ps, lhsT=wpc_t, rhs=cpi[:, k, :],
                                 start=False, stop=True)
                fps_k.append(f_ps)
            # fm slice = max over k (one PSUM input per op)
            fsl = fm_f32[:, pi * 128:(pi + 1) * 128]
            nc.scalar.activation(fsl, fps_k[0], AF.Copy)
            for kk in range(1, 4):
                nc.vector.tensor_tensor(out=fsl, in0=fsl, in1=fps_k[kk],
                                        op=ALU.max)
            # points slice: sum of 4 bf16 gathers in f32
            a01 = sb.tile([128, CC, 128], F32, tag="a01")
            nc.vector.tensor_tensor(out=a01, in0=gks[0], in1=gks[1], op=ALU.add)
            a23 = sb.tile([128, CC, 128], F32, tag="a23")
            nc.gpsimd.tensor_tensor(out=a23, in0=gks[2], in1=gks[3], op=ALU.add)
            nc.vector.tensor_tensor(out=pts_r[:, :, pi * 128:(pi + 1) * 128],
                                    in0=a01, in1=a23, op=ALU.add)

        # affinity/topk per pi; staging + gathers in two pi-pair batches so
        # gather descgen/data hide under the x-load tail; fuse last
        g2all = pers.tile([128, 128], I16, tag="g2all")

        for pi in range(4):
            emit_affinity_topk_stage(pi)
        nc.gpsimd.dma_start(out=GIDXv, in_=gstage)
        nc.scalar.dma_start(out=CROWv, in_=rowst)
        nc.scalar.dma_start(out=CCOLv, in_=colst)
        for g in range(8):
            nc.gpsimd.dma_start(out=g2all[16 * g:16 * (g + 1), :], in_=GIDXw)
        for pi in range(4):
            for k in range(K):
                gkT = gkp.tile([128, CC, 128], BF16, tag="gk")
                blk = (4 * k + pi) * 8
                nc.gpsimd.dma_gather(out_ap=gkT, in_ap=X_SB,
                                     idxs_ap=g2all[:, blk:blk + 8],
                                     num_idxs=128, num_idxs_reg=128,
                                     elem_size=C, transpose=True)
                gk_all[pi][k] = gkT
        for pi in range(4):
            emit_fuse_pts(pi)

        fm = pers.tile([128, P], BF16, tag="fm")
        nc.vector.tensor_scalar(fm, fm_f32, b2_t, None, op0=ALU.add)

        # conv over t (3 taps) + bias3 + relu -> tc bf16
        tc_bf = pers.tile([128, CC, P], BF16, tag="tcbf")
        for ct in range(CC):
            c_ps = ps.tile([128, P], F32, tag="acc")
            cs = slice(ct * 128, (ct + 1) * 128)
            nc.tensor.matmul(c_ps, lhsT=wtT_t[:, 1, cs], rhs=fm,
                             start=True, stop=False)
            nc.tensor.matmul(c_ps[:, R:P], lhsT=wtT_t[:, 0, cs], rhs=fm[:, 0:P - R],
                             start=False, stop=False)
            nc.tensor.matmul(c_ps[:, 0:P - R], lhsT=wtT_t[:, 2, cs], rhs=fm[:, R:P],
                             start=False, stop=True)
            nc.scalar.activation(tc_bf[:, ct, :], c_ps, AF.Relu,
                                 bias=b3_t[:, ct:ct + 1])
        tcT = pers.tile([128, CC, C], BF16, tag="tcT")
        for pb in range(4):
            tp2 = pstp.tile([128, C], BF16, tag="tp")
            for cc in range(CC):
                nc.tensor.transpose(tp2[:, cc * 128:(cc + 1) * 128],
                                    tc_bf[:, cc, pb * 128:(pb + 1) * 128], idb_t)
            nc.vector.tensor_copy(tcT[:, pb, :], tp2)

        # ---- phases 4+5: software-pipelined (skew=2) main loop ----
        SKEW = 3
        pj_tiles = [None] * NST
        projTP = {}

        def emit_zT_softmax(st):
            z_ps = ps.tile([128, P], F32, tag="acc")
            for cc in range(CC):
                nc.tensor.matmul(z_ps,
                                 lhsT=xcr[:, cc, st * 128:(st + 1) * 128],
                                 rhs=pts_r[:, cc, :],
                                 start=(cc == 0), stop=(cc == CC - 1))
            nm = sb3.tile([128, 1], F32, tag="nm")
            nc.vector.tensor_reduce(nm, z_ps, axis=AX.X, op=ALU.max, negate=True)
            nm4 = sb3.tile([128, 1], F32, tag="nm4")
            nc.vector.tensor_scalar(nm4, nm, 0.25, None, op0=ALU.mult)
            e_sb = sb3.tile([128, P], F32, tag="esb")
            dsum = sb3.tile([128, 1], F32, tag="dsum")
            nc.scalar.activation(e_sb, z_ps, AF.Exp, bias=nm4, scale=0.25,
                                 accum_out=dsum)
            rd = sb3.tile([128, 1], F32, tag="rd")
            nc.vector.reciprocal(rd, dsum)
            pjT = sb4.tile([128, P], BF16, tag="pjT")
            nc.vector.tensor_scalar(pjT, e_sb, rd, None, op0=ALU.mult)
            pj_tiles[st] = pjT

        def emit_transposes(st):
            chunk, slot = st // 4, st % 4
            if slot == 0:
                tiles = []
                for _pb in range(4):
                    pjt_tile = pstp.tile([128, P], BF16, tag="tp")
                    tiles.append(pjt_tile)
                projTP[chunk] = tiles
            tgt = projTP[chunk]
            pjT = pj_tiles[st]
            for pb in range(4):
                nc.tensor.transpose(tgt[pb][:, slot * 128:(slot + 1) * 128],
                                    pjT[:, pb * 128:(pb + 1) * 128], idb_t)

        def emit_prop(chunk):
            nslots = min(4, NST - chunk * 4)
            cw = nslots * 128
            tgt = projTP[chunk]
            proj_ch = sb.tile([128, 4, P], BF16, tag="projch")
            for pb in range(4):
                if pb < 2:
                    nc.scalar.activation(proj_ch[:, pb, 0:cw],
                                         tgt[pb][:, 0:cw], AF.Copy)
                else:
                    nc.vector.tensor_copy(proj_ch[:, pb, 0:cw],
                                          tgt[pb][:, 0:cw])
            for ct in range(CC):
                p_ps = pstp.tile([128, cw], F32, tag="tp")
                for pb in range(4):
                    nc.tensor.matmul(p_ps, lhsT=tcT[:, pb, ct * 128:(ct + 1) * 128],
                                     rhs=proj_ch[:, pb, 0:cw],
                                     start=(pb == 0), stop=(pb == 3))
                osb = sb4.tile([128, cw], F32, tag="osb")
                xsl = xc[:, ct, chunk * P:chunk * P + cw]
                if ct % 2 == 0:
                    nc.vector.tensor_tensor(out=osb, in0=p_ps, in1=xsl,
                                            op=ALU.add)
                else:
                    # GpSimd can't read PSUM: ACT copies, GpSimd adds in SBUF
                    nc.scalar.activation(osb, p_ps, AF.Copy)
                    nc.gpsimd.tensor_tensor(out=osb, in0=osb, in1=xsl,
                                            op=ALU.add)
                nc.sync.dma_start(out=OUTr[:, ct, chunk * P:chunk * P + cw],
                                  in_=osb)

        for st in range(NST):
            emit_zT_softmax(st)
            if st >= SKEW:
                emit_transposes(st - SKEW)
                if (st - SKEW) % 4 == 3:
                    emit_prop((st - SKEW) // 4)
        for st in range(NST - SKEW, NST):
            emit_transposes(st)
            if st % 4 == 3 or st == NST - 1:
                emit_prop(st // 4)
        ctx.close()
    nc.compile()
    return nc


def _host_prep(inputs):
    eps = 1e-5
    f32 = np.float32
    x = np.asarray(inputs["input"], f32)                       # (B,C,T,H,W)
    s1 = np.asarray(inputs["bn1_gamma"]) / np.sqrt(np.asarray(inputs["bn1_var"]) + eps)
    wrT = (np.asarray(inputs["w_reduce"], f32) * s1[:, None]).T.astype(f32)
    s2 = np.asarray(inputs["bn2_gamma"]) / np.sqrt(np.asarray(inputs["bn2_var"]) + eps)
    wp = np.asarray(inputs["w_proj"], f32) * s2[:, None]       # (Cq, C+2)
    b2 = (np.asarray(inputs["bn2_beta"])
          - np.asarray(inputs["bn2_mean"]) * s2).astype(f32)
    s3 = np.asarray(inputs["bn3_gamma"]) / np.sqrt(np.asarray(inputs["bn3_var"]) + eps)
    wt = np.asarray(inputs["w_t"], f32)[:, :, :, 0] * s3[:, None, None]  # (C,Cq,3)
    b3 = (np.asarray(inputs["bn3_beta"])
          - np.asarray(inputs["bn3_mean"]) * s3).astype(f32)
    import ml_dtypes
    bf16 = ml_dtypes.bfloat16
    common = {
        "wrT": np.ascontiguousarray(wrT),
        "wpT": np.ascontiguousarray(wp[:, :C].T.astype(bf16)),
        "wpc": np.ascontiguousarray(wp[:, C:].T.astype(bf16)),
        "wtT": np.ascontiguousarray(np.transpose(wt, (2, 1, 0)).astype(bf16)),
        "b2": b2.reshape(Cq, 1),
        "b3": b3.reshape(CC, 128),
        "identf": np.eye(128, dtype=f32),
        "identbf": np.eye(128, dtype=bf16),
    }
    iota = np.arange(128, dtype=f32)
    common["iota7"] = np.ascontiguousarray(
        np.stack([iota + 128 * hb for hb in range(7)], axis=1))
    common["ones1"] = np.ones((1, 128), f32)
    x_cs = x.reshape(B, C, S)
    x_sc = np.ascontiguousarray(np.transpose(x_cs, (0, 2, 1)))
    x_sc_bf = x_sc.astype(bf16)
    x_sc0 = np.zeros((B, 896, C), f32)
    x_sc0[:, 0:HW] = x_sc[:, 0:HW]
    in_maps = []
    for b in range(B):
        m = dict(common)
        m["x_cs"] = np.ascontiguousarray(x_cs[b])
        m["x_sc0"] = x_sc0[b]
        m["x_sc_bf"] = x_sc_bf[b]
        in_maps.append(m)
    return in_maps


def kernel(**inputs) -> np.ndarray:
    if "nc" not in _CACHED:
        _CACHED["nc"] = build_nc()
    nc = _CACHED["nc"]
    in_maps = _host_prep(inputs)
    res = run_bass_kernel_spmd(nc, in_maps, list(range(B)))
    out = np.stack([res.results[b]["out"] for b in range(B)], axis=0)
    return out.reshape(B, C, T, H, W).astype(np.float32)


if __name__ == "__main__":
    pass
